# revision 19
# baseline (speedup 1.0000x reference)
"""Trainium2 Bass kernel for multi-head attention (B=4, N=2048, DIM=1024, H=16, DH=64).

Sharding (head-parallel + row-parallel to_out): 8 cores = 4 batches x 2 head-halves.
Each core computes q/k/v for its 8 heads over the FULL 2048-token sequence (no
duplicated projection work, unlike query-split sharding), runs attention for those
heads, and its row-parallel half of the output projection. The to_out all-reduce
happens on the host at gather time: out[b] = partial[core 2b] + partial[core 2b+1]
(bias is added on the even core only; odd cores receive a zero bias input).

The attention phase is ACT(exp)-bound (~1147 ns per [128,1024] exp vs ~860 ns PE
work per step), so projections are hoisted into a DMA-racing pre-phase and the
output projection into two dense PE blocks. Softmax denominators are folded into
the AV matmul via a ones-column in V; the reciprocal uses reciprocal_approx_fast
(single DVE pass, ~5x faster) + DRAM round-trip partition broadcast.
"""

import numpy as np
import ml_dtypes

import concourse.bass as bass
import concourse.tile as tile
from concourse import bacc, mybir
from concourse import bass_utils

B, N, DIM = 4, 2048, 1024
HEADS_TOT, DH = 16, 64
SCALE = DH ** -0.5
NCORES = 8

HPC = 8              # heads per core
NPAIR = HPC // 2     # head-pairs per core = 4
KT = DIM // 128      # 8 contraction tiles
NT = N // 128        # 16 key tiles
IC = 2               # query blocks
IB = N // IC         # 1024 queries per block
BF16 = mybir.dt.bfloat16
F32 = mybir.dt.float32

_CACHE = {}


def _build_program():
    nc = bacc.Bacc("TRN2", target_bir_lowering=False, debug=False)

    xT_d = nc.dram_tensor("xT", [128, KT, N], BF16, kind="ExternalInput")
    w_d = nc.dram_tensor("w_qkv", [128, 3, KT, 512], BF16, kind="ExternalInput")
    wo_d = nc.dram_tensor("w_out", [128, NPAIR, DIM], BF16, kind="ExternalInput")
    bout_d = nc.dram_tensor("b_out", [DIM], F32, kind="ExternalInput")
    out_d = nc.dram_tensor("out", [N, DIM], F32, kind="ExternalOutput")

    with tile.TileContext(nc) as tc:
        _emit(tc, nc, xT_d, w_d, wo_d, bout_d, out_d)
    nc.compile()
    return nc


def _emit(tc, nc, xT_d, w_d, wo_d, bout_d, out_d):
    from contextlib import ExitStack

    bap = bout_d.ap()
    bias_bcast = bass.AP(tensor=bap.tensor, offset=bap.offset,
                         ap=[[0, 128]] + [list(d) for d in bap.ap])

    with ExitStack() as ctx:
        consts = ctx.enter_context(tc.tile_pool(name="consts", bufs=1))
        stage = ctx.enter_context(tc.tile_pool(name="stage", bufs=1))
        qkv = ctx.enter_context(tc.tile_pool(name="qkv", bufs=1))
        ao = ctx.enter_context(tc.tile_pool(name="ao", bufs=1))
        atp = ctx.enter_context(tc.tile_pool(name="atp", bufs=4))
        avup = ctx.enter_context(tc.tile_pool(name="avu", bufs=2))
        rcp = ctx.enter_context(tc.tile_pool(name="rcp", bufs=2))
        bcsp = ctx.enter_context(tc.tile_pool(name="bcs", bufs=2))
        oddp = ctx.enter_context(tc.tile_pool(name="odd", bufs=2))
        drbp = ctx.enter_context(tc.tile_pool(name="drb", bufs=2, space="DRAM"))
        stp = ctx.enter_context(tc.tile_pool(name="stp", bufs=2))

        # ---- constants / weights DMA ----
        bias_sb = consts.tile([128, DIM], F32)
        nc.scalar.dma_start(out=bias_sb, in_=bias_bcast)
        wo_sb = consts.tile([128, NPAIR, DIM], BF16)
        nc.scalar.dma_start(out=wo_sb, in_=wo_d.ap())

        # preload exp table set early (off the critical path)
        at_warm = consts.tile([128, 1], BF16)
        nc.scalar.activation(out=at_warm, in_=bias_sb[:, 0:1],
                             func=mybir.ActivationFunctionType.Exp)

        wv = stage.tile([128, KT, 512], BF16)
        wk = stage.tile([128, KT, 512], BF16)
        wq = stage.tile([128, KT, 512], BF16)
        nc.scalar.dma_start(out=wv, in_=w_d.ap()[:, 0])
        nc.scalar.dma_start(out=wk, in_=w_d.ap()[:, 1])
        nc.scalar.dma_start(out=wq, in_=w_d.ap()[:, 2])

        xTk = [stage.tile([128, N], BF16, name=f"xTk{k}") for k in range(KT)]
        for k in range(KT):
            eng = nc.sync if k % 2 == 0 else nc.gpsimd
            eng.dma_start(out=xTk[k], in_=xT_d.ap()[:, k, :])

        # ---- persistent qkv / attention-out tiles ----
        kTs = [qkv.tile([128, N], BF16, name=f"kT{s}") for s in range(NPAIR)]
        qTs = [qkv.tile([128, N], BF16, name=f"qT{s}") for s in range(NPAIR)]
        vt = qkv.tile([128, NT, HPC, DH + 1], BF16)
        nc.vector.memset(vt[:, :, :, DH], 1.0)
        aoTs = [ao.tile([128, N], BF16, name=f"aoT{s}") for s in range(NPAIR)]

        # ---- phase 1: all projections, racing the input DMAs ----
        with tc.tile_pool(name="pre_ps", bufs=4, space="PSUM") as prep:
            def pre_pool():
                return (prep, "p")

            def vchunk(t):
                pool, tag = pre_pool()
                ps = pool.tile([128, 512], F32, tag=tag, name=f"vps{t}")
                for kt in range(KT):
                    nc.tensor.matmul(ps, xTk[kt][:, 128 * t:128 * (t + 1)],
                                     wv[:, kt, :],
                                     start=(kt == 0), stop=(kt == KT - 1))
                nc.vector.tensor_copy(
                    out=vt[:, t, :, 0:DH],
                    in_=ps.rearrange("p (h d) -> p h d", h=HPC))

            def kqchunk(w, dst, s, c):
                pool, tag = pre_pool()
                ps = pool.tile([128, 512], F32, tag=tag, name=f"ps{s}_{c}")
                for kt in range(KT):
                    nc.tensor.matmul(ps, w[:, kt, 128 * s:128 * (s + 1)],
                                     xTk[kt][:, 512 * c:512 * (c + 1)],
                                     start=(kt == 0), stop=(kt == KT - 1))
                nc.vector.tensor_copy(out=dst[s][:, 512 * c:512 * (c + 1)], in_=ps)

            for t in range(NT):
                vchunk(t)
            for s in range(NPAIR):
                for c in range(4):
                    kqchunk(wk, kTs, s, c)
            for s in range(NPAIR):
                for c in range(4):
                    kqchunk(wq, qTs, s, c)

        scp = ctx.enter_context(tc.tile_pool(name="scp", bufs=2, space="PSUM"))
        avp = ctx.enter_context(tc.tile_pool(name="avp", bufs=2, space="PSUM"))

        def normalize(s, ic, p, av):
            sfx = f"{s}_{ic}_{p}"
            avu = avup.tile([DH + 1, IB], F32, tag="avu", name=f"avu{sfx}")
            nc.vector.tensor_copy(out=avu, in_=av)
            rc = rcp.tile([DH + 1, IB], F32, tag="rc", name=f"rc{sfx}")
            nc.vector.reciprocal(out=rc[DH:DH + 1, :], in_=avu[DH:DH + 1, :])
            dr = drbp.tile([IB], F32, tag="dr", name=f"dr{sfx}")
            nc.sync.dma_start(out=dr, in_=rc[DH:DH + 1, :])
            dr_bc = bass.AP(tensor=dr.tensor, offset=dr.offset,
                            ap=[[0, DH]] + [list(dd) for dd in dr.ap])
            bcs = bcsp.tile([DH, IB], F32, tag="bcs", name=f"bcs{sfx}")
            nc.sync.dma_start(out=bcs, in_=dr_bc)
            if p == 0:
                with nc.allow_low_precision(reason="attn out in bf16"):
                    nc.vector.tensor_mul(
                        out=aoTs[s][0:DH, IB * ic:IB * (ic + 1)],
                        in0=avu[0:DH, :], in1=bcs)
            else:
                od = oddp.tile([DH, IB], BF16, tag="od", name=f"od{s}_{ic}")
                with nc.allow_low_precision(reason="attn out in bf16"):
                    nc.vector.tensor_mul(out=od, in0=avu[0:DH, :], in1=bcs)
                nc.gpsimd.dma_start(
                    out=aoTs[s][DH:128, IB * ic:IB * (ic + 1)], in_=od)

        def block(s, ic):
            av0 = avp.tile([DH + 1, IB], F32, tag="av", name=f"av0_{s}_{ic}")
            av1 = avp.tile([DH + 1, IB], F32, tag="av", name=f"av1_{s}_{ic}")
            avs = [av0, av1]
            for t in range(NT):
                for p in range(2):
                    pb = 64 * p
                    sc = scp.tile([128, IB], F32, tag="sc", name=f"sc{s}_{ic}_{t}_{p}")
                    for c in range(2):
                        nc.tensor.matmul(
                            sc[:, 512 * c:512 * (c + 1)],
                            kTs[s][pb:pb + 64, 128 * t:128 * (t + 1)],
                            qTs[s][pb:pb + 64,
                                   IB * ic + 512 * c:IB * ic + 512 * (c + 1)],
                            start=True, stop=True, tile_position=(pb, 0))
                    at = atp.tile([128, IB], BF16, tag="at",
                                  name=f"at{s}_{ic}_{t}_{p}")
                    nc.scalar.activation(out=at, in_=sc,
                                         func=mybir.ActivationFunctionType.Exp,
                                         scale=SCALE)
                    h = 2 * s + p
                    for c in range(2):
                        nc.tensor.matmul(
                            avs[p][:, 512 * c:512 * (c + 1)],
                            vt[:, t, h, :],
                            at[:, 512 * c:512 * (c + 1)],
                            start=(t == 0), stop=(t == NT - 1))
            for p in range(2):
                normalize(s, ic, p, avs[p])

        def oproj(ns):
            po = scp.tile([128, DIM], F32, tag="sc", name=f"po{ns}")
            for c in range(2):
                for hp in range(NPAIR):
                    nc.tensor.matmul(
                        po[:, 512 * c:512 * (c + 1)],
                        aoTs[hp][:, 128 * ns:128 * (ns + 1)],
                        wo_sb[:, hp, 512 * c:512 * (c + 1)],
                        start=(hp == 0), stop=(hp == NPAIR - 1))
            st = stp.tile([128, DIM], F32, tag="st", name=f"st{ns}")
            nc.vector.tensor_add(out=st, in0=po, in1=bias_sb)
            nc.sync.dma_start(out=out_d.ap()[128 * ns:128 * (ns + 1), :], in_=st)

        # ---- attention + output projection ----
        for s in range(NPAIR):
            block(s, 0)
        for s in range(NPAIR):
            block(s, 1)
            if s == 3:
                for ns in range(0, 8):
                    oproj(ns)
        for ns in range(8, 16):
            oproj(ns)


def get_program():
    if "nc" not in _CACHE:
        _CACHE["nc"] = _build_program()
    return _CACHE["nc"]


def make_in_maps(x, w_qkv, w_out, b_out):
    bf = ml_dtypes.bfloat16
    x = np.asarray(x, np.float32)
    w_qkv = np.asarray(w_qkv, np.float32)
    w_out = np.asarray(w_out, np.float32)
    b_out = np.asarray(b_out, np.float32)
    zeros_b = np.zeros_like(b_out)

    in_maps = []
    for core in range(NCORES):
        b, hh = core // 2, core % 2
        # xT in [128, KT, N] layout: [p, t, n] = x[b].T[t*128+p, n]
        xT = np.ascontiguousarray(x[b].T).astype(bf)                 # [DIM, N]
        xT_pt = np.ascontiguousarray(xT.reshape(KT, 128, N).transpose(1, 0, 2))
        # w slices for this head-half, groups ordered [v, k, q]
        wq = w_qkv[:, 512 * hh:512 * (hh + 1)]
        wk = w_qkv[:, DIM + 512 * hh:DIM + 512 * (hh + 1)]
        wv = w_qkv[:, 2 * DIM + 512 * hh:2 * DIM + 512 * (hh + 1)]
        wcat = np.stack([wv, wk, wq], axis=0).astype(bf)             # [3, DIM, 512]
        w_pt = np.ascontiguousarray(
            wcat.reshape(3, KT, 128, 512).transpose(2, 0, 1, 3))    # [p, g, t, e]
        # w_out rows for this half -> [p, hp, d]
        wo = w_out[512 * hh:512 * (hh + 1), :].astype(bf)            # [512, DIM]
        wo_pt = np.ascontiguousarray(wo.reshape(NPAIR, 128, DIM).transpose(1, 0, 2))
        in_maps.append({
            "xT": xT_pt,
            "w_qkv": w_pt,
            "w_out": wo_pt,
            "b_out": b_out if hh == 0 else zeros_b,
        })
    return in_maps


def kernel(x, w_qkv, w_out, b_out):
    nc = get_program()
    in_maps = make_in_maps(x, w_qkv, w_out, b_out)
    res = bass_utils.run_bass_kernel_spmd(nc, in_maps, core_ids=list(range(NCORES)))
    out = np.empty((B, N, DIM), np.float32)
    for b in range(B):
        out[b] = res.results[2 * b]["out"]
        out[b] += res.results[2 * b + 1]["out"]
    return out


# revision 20
# speedup vs baseline: 1.2271x; 1.2271x over previous
"""Trainium2 Bass kernel for multi-head attention (B=4, N=2048, DIM=1024, H=16, DH=64).

Sharding (head-parallel + row-parallel to_out): 8 cores = 4 batches x 2 head-halves.
Each core computes q/k/v for its 8 heads over the FULL 2048-token sequence (no
duplicated projection work, unlike query-split sharding), runs attention for those
heads, and its row-parallel half of the output projection. The to_out all-reduce
happens on the host at gather time: out[b] = partial[core 2b] + partial[core 2b+1]
(bias is added on the even core only; odd cores receive a zero bias input).

The attention phase is ACT(exp)-bound (~1147 ns per [128,1024] exp vs ~860 ns PE
work per step), so projections are hoisted into a DMA-racing pre-phase and the
output projection into two dense PE blocks. Softmax denominators are folded into
the AV matmul via a ones-column in V; the reciprocal uses reciprocal_approx_fast
(single DVE pass, ~5x faster) + DRAM round-trip partition broadcast.
"""

import numpy as np
import ml_dtypes

import concourse.bass as bass
import concourse.tile as tile
from concourse import bacc, mybir
from concourse import bass_utils

B, N, DIM = 4, 2048, 1024
HEADS_TOT, DH = 16, 64
SCALE = DH ** -0.5
NCORES = 8

HPC = 8              # heads per core
NPAIR = HPC // 2     # head-pairs per core = 4
KT = DIM // 128      # 8 contraction tiles
NT = N // 128        # 16 key tiles
IC = 2               # query blocks
IB = N // IC         # 1024 queries per block
BF16 = mybir.dt.bfloat16
F32 = mybir.dt.float32

_CACHE = {}


def _build_program():
    nc = bacc.Bacc("TRN2", target_bir_lowering=False, debug=False)

    xT_d = nc.dram_tensor("xT", [128, KT, N], BF16, kind="ExternalInput")
    w_d = nc.dram_tensor("w_qkv", [128, 3, KT, 512], BF16, kind="ExternalInput")
    wo_d = nc.dram_tensor("w_out", [128, NPAIR, DIM], BF16, kind="ExternalInput")
    bout_d = nc.dram_tensor("b_out", [DIM], F32, kind="ExternalInput")
    out_d = nc.dram_tensor("out", [N, DIM], BF16, kind="ExternalOutput")

    with tile.TileContext(nc) as tc:
        _emit(tc, nc, xT_d, w_d, wo_d, bout_d, out_d)
    nc.compile()
    return nc


def _emit(tc, nc, xT_d, w_d, wo_d, bout_d, out_d):
    from contextlib import ExitStack

    bap = bout_d.ap()
    bias_bcast = bass.AP(tensor=bap.tensor, offset=bap.offset,
                         ap=[[0, 128]] + [list(d) for d in bap.ap])

    with ExitStack() as ctx:
        consts = ctx.enter_context(tc.tile_pool(name="consts", bufs=1))
        stage = ctx.enter_context(tc.tile_pool(name="stage", bufs=1))
        qkv = ctx.enter_context(tc.tile_pool(name="qkv", bufs=1))
        ao = ctx.enter_context(tc.tile_pool(name="ao", bufs=1))
        atp = ctx.enter_context(tc.tile_pool(name="atp", bufs=4))
        avup = ctx.enter_context(tc.tile_pool(name="avu", bufs=2))
        rcp = ctx.enter_context(tc.tile_pool(name="rcp", bufs=2))
        bcsp = ctx.enter_context(tc.tile_pool(name="bcs", bufs=2))
        oddp = ctx.enter_context(tc.tile_pool(name="odd", bufs=2))
        drbp = ctx.enter_context(tc.tile_pool(name="drb", bufs=2, space="DRAM"))
        stp = ctx.enter_context(tc.tile_pool(name="stp", bufs=2))

        # ---- constants / weights DMA (wv first: it gates the first matmuls) ----
        bias_sb = consts.tile([128, DIM], F32)
        wo_sb = consts.tile([128, NPAIR, DIM], BF16)
        at_warm = consts.tile([128, 32], BF16)
        wv = stage.tile([128, KT, 512], BF16)
        wk = stage.tile([128, KT, 512], BF16)
        wq = stage.tile([128, KT, 512], BF16)
        nc.scalar.dma_start(out=wv, in_=w_d.ap()[:, 0])

        # preload exp table set early (off the critical path)
        nc.scalar.activation(out=at_warm, in_=wv[:, 0, 0:32],
                             func=mybir.ActivationFunctionType.Exp)

        nc.scalar.dma_start(out=wk, in_=w_d.ap()[:, 1])
        nc.scalar.dma_start(out=wq, in_=w_d.ap()[:, 2])
        nc.scalar.dma_start(out=bias_sb, in_=bias_bcast)
        nc.scalar.dma_start(out=wo_sb, in_=wo_d.ap())

        xTk = [stage.tile([128, N], BF16, name=f"xTk{k}") for k in range(KT)]
        for k in range(KT):
            eng = nc.sync if k % 2 == 0 else nc.gpsimd
            eng.dma_start(out=xTk[k], in_=xT_d.ap()[:, k, :])

        # ---- persistent qkv / attention-out tiles ----
        kTs = [qkv.tile([128, N], BF16, name=f"kT{s}") for s in range(NPAIR)]
        qTs = [qkv.tile([128, N], BF16, name=f"qT{s}") for s in range(NPAIR)]
        vt = qkv.tile([128, NT, HPC, DH + 1], BF16)
        nc.vector.memset(vt[:, :, :, DH], 1.0)
        aoTs = [ao.tile([128, N], BF16, name=f"aoT{s}") for s in range(NPAIR)]

        # ---- phase 1: all projections, racing the input DMAs ----
        with tc.tile_pool(name="pre_ps", bufs=4, space="PSUM") as prep:
            def pre_pool():
                return (prep, "p")

            def vchunk(t):
                pool, tag = pre_pool()
                ps = pool.tile([128, 512], F32, tag=tag, name=f"vps{t}")
                for kt in range(KT):
                    nc.tensor.matmul(ps, xTk[kt][:, 128 * t:128 * (t + 1)],
                                     wv[:, kt, :],
                                     start=(kt == 0), stop=(kt == KT - 1))
                nc.vector.tensor_copy(
                    out=vt[:, t, :, 0:DH],
                    in_=ps.rearrange("p (h d) -> p h d", h=HPC))

            def kqchunk(w, dst, s, c):
                pool, tag = pre_pool()
                ps = pool.tile([128, 512], F32, tag=tag, name=f"ps{s}_{c}")
                for kt in range(KT):
                    nc.tensor.matmul(ps, w[:, kt, 128 * s:128 * (s + 1)],
                                     xTk[kt][:, 512 * c:512 * (c + 1)],
                                     start=(kt == 0), stop=(kt == KT - 1))
                nc.vector.tensor_copy(out=dst[s][:, 512 * c:512 * (c + 1)], in_=ps)

            for t in range(NT):
                vchunk(t)
            for s in range(NPAIR):
                for c in range(4):
                    kqchunk(wk, kTs, s, c)
            for s in range(NPAIR):
                for c in range(4):
                    kqchunk(wq, qTs, s, c)

        scp = ctx.enter_context(tc.tile_pool(name="scp", bufs=2, space="PSUM"))
        avp = ctx.enter_context(tc.tile_pool(name="avp", bufs=2, space="PSUM"))

        def normalize(s, ic, p, av):
            sfx = f"{s}_{ic}_{p}"
            avu = avup.tile([DH + 1, IB], F32, tag="avu", name=f"avu{sfx}")
            nc.vector.tensor_copy(out=avu, in_=av)
            # spread the denominator over 64 partitions so the reciprocal runs
            # 64-wide (SBUF->SBUF DMA partition restructure) instead of 6.5us
            # on a single partition
            dsp = rcp.tile([DH, IB // DH], F32, tag="dsp", name=f"dsp{sfx}")
            nc.sync.dma_start(out=dsp, in_=avu[DH:DH + 1, :])
            rc = rcp.tile([DH, IB // DH], F32, tag="rc", name=f"rc{sfx}")
            nc.vector.reciprocal(out=rc, in_=dsp)
            dr = drbp.tile([IB], F32, tag="dr", name=f"dr{sfx}")
            dr_sq = bass.AP(tensor=dr.tensor, offset=dr.offset,
                            ap=[[IB // DH, DH], [1, IB // DH]])
            nc.sync.dma_start(out=dr_sq, in_=rc)
            dr_bc = bass.AP(tensor=dr.tensor, offset=dr.offset,
                            ap=[[0, DH]] + [list(dd) for dd in dr.ap])
            bcs = bcsp.tile([DH, IB], F32, tag="bcs", name=f"bcs{sfx}")
            nc.sync.dma_start(out=bcs, in_=dr_bc)
            if p == 0:
                with nc.allow_low_precision(reason="attn out in bf16"):
                    nc.vector.tensor_mul(
                        out=aoTs[s][0:DH, IB * ic:IB * (ic + 1)],
                        in0=avu[0:DH, :], in1=bcs)
            else:
                od = oddp.tile([DH, IB], BF16, tag="od", name=f"od{s}_{ic}")
                with nc.allow_low_precision(reason="attn out in bf16"):
                    nc.vector.tensor_mul(out=od, in0=avu[0:DH, :], in1=bcs)
                nc.gpsimd.dma_start(
                    out=aoTs[s][DH:128, IB * ic:IB * (ic + 1)], in_=od)

        def block(s, ic, warm=False):
            av0 = avp.tile([DH + 1, IB], F32, tag="av", name=f"av0_{s}_{ic}")
            av1 = avp.tile([DH + 1, IB], F32, tag="av", name=f"av1_{s}_{ic}")
            avs = [av0, av1]
            for t in range(NT):
                for p in range(2):
                    pb = 64 * p
                    sc = scp.tile([128, IB], F32, tag="sc", name=f"sc{s}_{ic}_{t}_{p}")
                    if not (t == 0 and p == 0):
                        # continuous HAM warm filler: one redundant score MM per
                        # step fills the PE slack while ACT runs exp; keeps PE
                        # duty high so a cold clock-gate state self-heals
                        nc.tensor.matmul(
                            sc[:, 0:512],
                            kTs[s][pb:pb + 64, 128 * t:128 * (t + 1)],
                            qTs[s][pb:pb + 64,
                                   IB * ic + 512 * 0:IB * ic + 512 * 1],
                            start=True, stop=True, tile_position=(pb, 0))
                    if warm and t == 0 and p == 0:
                        # HAM warm-guard: ~8 redundant back-to-back score MMs give
                        # the PE a >=3.4us sustained-busy window at phase entry, so
                        # a cold (K=4/8) clock-gate state cannot stick for the
                        # whole ACT-bound phase. Overwritten by the real MMs below.
                        for _ in range(8):
                            nc.tensor.matmul(
                                sc[:, 0:512],
                                kTs[s][pb:pb + 64, 128 * t:128 * (t + 1)],
                                qTs[s][pb:pb + 64, IB * ic:IB * ic + 512],
                                start=True, stop=True, tile_position=(pb, 0))
                    for c in range(2):
                        nc.tensor.matmul(
                            sc[:, 512 * c:512 * (c + 1)],
                            kTs[s][pb:pb + 64, 128 * t:128 * (t + 1)],
                            qTs[s][pb:pb + 64,
                                   IB * ic + 512 * c:IB * ic + 512 * (c + 1)],
                            start=True, stop=True, tile_position=(pb, 0))
                    at = atp.tile([128, IB], BF16, tag="at",
                                  name=f"at{s}_{ic}_{t}_{p}")
                    nc.scalar.activation(out=at, in_=sc,
                                         func=mybir.ActivationFunctionType.Exp,
                                         scale=SCALE)
                    h = 2 * s + p
                    for c in range(2):
                        nc.tensor.matmul(
                            avs[p][:, 512 * c:512 * (c + 1)],
                            vt[:, t, h, :],
                            at[:, 512 * c:512 * (c + 1)],
                            start=(t == 0), stop=(t == NT - 1))
            for p in range(2):
                normalize(s, ic, p, avs[p])

        def oproj(ns):
            po = scp.tile([128, DIM], F32, tag="sc", name=f"po{ns}")
            for c in range(2):
                for hp in range(NPAIR):
                    nc.tensor.matmul(
                        po[:, 512 * c:512 * (c + 1)],
                        aoTs[hp][:, 128 * ns:128 * (ns + 1)],
                        wo_sb[:, hp, 512 * c:512 * (c + 1)],
                        start=(hp == 0), stop=(hp == NPAIR - 1))
            st = stp.tile([128, DIM], BF16, tag="st", name=f"st{ns}")
            with nc.allow_low_precision(reason="bf16 partial output"):
                nc.vector.tensor_add(out=st, in0=po, in1=bias_sb)
            nc.sync.dma_start(out=out_d.ap()[128 * ns:128 * (ns + 1), :], in_=st)

        # ---- attention + output projection ----
        for s in range(NPAIR):
            block(s, 0, warm=(s == 0))
        for ns in range(0, 8):           # out rows of query block 0 (mid-phase)
            oproj(ns)
        for s in range(NPAIR):
            block(s, 1, warm=(s == 0))
        for ns in range(8, 16):
            oproj(ns)


def get_program():
    if "nc" not in _CACHE:
        _CACHE["nc"] = _build_program()
    return _CACHE["nc"]


def make_in_maps(x, w_qkv, w_out, b_out):
    bf = ml_dtypes.bfloat16
    x = np.asarray(x, np.float32)
    w_qkv = np.asarray(w_qkv, np.float32)
    w_out = np.asarray(w_out, np.float32)
    b_out = np.asarray(b_out, np.float32)
    zeros_b = np.zeros_like(b_out)

    in_maps = []
    for core in range(NCORES):
        b, hh = core // 2, core % 2
        # xT in [128, KT, N] layout: [p, t, n] = x[b].T[t*128+p, n]
        xT = np.ascontiguousarray(x[b].T).astype(bf)                 # [DIM, N]
        xT_pt = np.ascontiguousarray(xT.reshape(KT, 128, N).transpose(1, 0, 2))
        # w slices for this head-half, groups ordered [v, k, q]
        wq = w_qkv[:, 512 * hh:512 * (hh + 1)]
        wk = w_qkv[:, DIM + 512 * hh:DIM + 512 * (hh + 1)]
        wv = w_qkv[:, 2 * DIM + 512 * hh:2 * DIM + 512 * (hh + 1)]
        wcat = np.stack([wv, wk, wq], axis=0).astype(bf)             # [3, DIM, 512]
        w_pt = np.ascontiguousarray(
            wcat.reshape(3, KT, 128, 512).transpose(2, 0, 1, 3))    # [p, g, t, e]
        # w_out rows for this half -> [p, hp, d]
        wo = w_out[512 * hh:512 * (hh + 1), :].astype(bf)            # [512, DIM]
        wo_pt = np.ascontiguousarray(wo.reshape(NPAIR, 128, DIM).transpose(1, 0, 2))
        in_maps.append({
            "xT": xT_pt,
            "w_qkv": w_pt,
            "w_out": wo_pt,
            "b_out": b_out if hh == 0 else zeros_b,
        })
    return in_maps


def kernel(x, w_qkv, w_out, b_out):
    nc = get_program()
    in_maps = make_in_maps(x, w_qkv, w_out, b_out)
    res = bass_utils.run_bass_kernel_spmd(nc, in_maps, core_ids=list(range(NCORES)))
    out = np.empty((B, N, DIM), np.float32)
    for b in range(B):
        out[b] = np.asarray(res.results[2 * b]["out"], np.float32)
        out[b] += np.asarray(res.results[2 * b + 1]["out"], np.float32)
    return out
